# revision 16
# baseline (speedup 1.0000x reference)
"""bf16 1-D Winograd F(2,3) conv (along W), data-parallel over batch.

Per core: 4 images. Host precomputes the 1-D Winograd input transform
V_p (4 phases, [C,58,28] per image) and weight transform U (12 mats per
C_out half). Per (row-tile t of 14 rows, half h): 12 matmuls of
[128x128] x [128x392] build phase sums M0..M3 in 4 PSUM banks (3 kh
taps accumulated each); even cols = M0+M1+M2+b on DVE, odd cols =
M1-M2-M3+b via ACT psum->sbuf evacuation + GpSimd combines. 384 MMs of
392 cycles vs direct conv's 504 of 448 -> 1.5x fewer PE cycles.
"""

import sys

if "/opt/trn_rl_repo" not in sys.path:
    sys.path.insert(0, "/opt/trn_rl_repo")

import numpy as np

N, C_IN, H, W = 32, 128, 56, 56
C_OUT, KH, KW = 256, 3, 3
N_CORES = 8
IMGS = N // N_CORES
HP, WP = H + 2, W + 2
VT = W // 2            # 28 output-col pairs
RPT = 14               # output rows per tile
NT = H // RPT          # 4
TF = RPT * VT          # 392 free elements per matmul
NH = C_OUT // 128      # 2
NPH = 4                # winograd phases
N_WARMUP_MM = 10
POOL_UNITS = (1, 3, 5)  # of every 8 (t,h) units, these combine on gpsimd

_CACHE = {}


def _build_program():
    import concourse.mybir as mybir
    import concourse.tile as tile
    from concourse import bacc

    F32 = mybir.dt.float32
    BF16 = mybir.dt.bfloat16
    ADD = mybir.AluOpType.add
    SUB = mybir.AluOpType.subtract
    COPY = mybir.ActivationFunctionType.Copy
    IDENT = mybir.ActivationFunctionType.Identity

    nc = bacc.Bacc("TRN2", target_bir_lowering=False, debug=False,
                   enable_asserts=False)

    v = nc.dram_tensor("v", [IMGS, C_IN, NPH, HP, VT], BF16,
                       kind="ExternalInput").ap()
    w = nc.dram_tensor("w", [NH, C_IN, NPH * KH, 128], BF16,
                       kind="ExternalInput").ap()
    b = nc.dram_tensor("b", [128, NH], F32, kind="ExternalInput").ap()
    out = nc.dram_tensor("out", [IMGS, C_OUT, H, W], F32,
                         kind="ExternalOutput").ap()
    out_v = out.rearrange("n c a b -> n c (a b)")

    with tile.TileContext(nc) as tc:
        with (
            tc.tile_pool(name="consts", bufs=1) as consts,
            tc.tile_pool(name="vin", bufs=4) as vin,
            tc.tile_pool(name="evac", bufs=2) as evac,
            tc.tile_pool(name="tmpp", bufs=2) as tmpp,
            tc.tile_pool(name="outp", bufs=2) as outp,
            tc.tile_pool(name="psum", bufs=2, space="PSUM") as psum,
        ):
            w_sb = consts.tile([C_IN, NH, NPH * KH, 128], BF16, tag="w")
            b_sb = consts.tile([128, NH], F32, tag="b")

            # Two physical DMA rings, serial per ring, ~2.3us completion
            # latency per queue slot: img0's V arrives in consumption
            # order as [phases 0-1 rows<16, phases 2-3 rows<16, rows
            # 16:40 all phases, rows 40:58 all phases] on the sync ring
            # while w0/b/w1 ride the scalar ring; image prefetches fill
            # the remaining slots.
            v0 = vin.tile([C_IN, NPH, HP, VT], BF16, tag="v")
            nc.sync.dma_start(out=v0[:, :, 0:16], in_=v[0, :, :, 0:16])
            nc.scalar.dma_start(out=w_sb[:, 0], in_=w[0])
            nc.sync.dma_start(out=v0[:, :, 16:30], in_=v[0, :, :, 16:30])
            nc.scalar.dma_start(out=b_sb[:], in_=b)
            nc.sync.dma_start(out=v0[:, :, 30:44], in_=v[0, :, :, 30:44])
            nc.scalar.dma_start(out=w_sb[:, 1], in_=w[1])
            nc.sync.dma_start(out=v0[:, :, 44:HP], in_=v[0, :, :, 44:HP])

            scratch = consts.tile([128, TF], BF16, tag="scratch")
            nc.gpsimd.memset(scratch[:], 0.0)
            warm_ps = psum.tile([128, 512], F32, tag="M0")
            for _ in range(N_WARMUP_MM):
                nc.tensor.matmul(warm_ps[:, :TF], lhsT=scratch[:, :128],
                                 rhs=scratch[:, :], start=True, stop=True)

            # later images' V tiles; their prefetch dma_starts are
            # deferred into the unit loop so the transfers never contend
            # with img0's critical slabs for HBM (vin bufs=4 keeps the
            # dma_starts free of blocking waits).
            vts = {0: v0}
            for img in range(1, IMGS):
                vts[img] = vin.tile([C_IN, NPH, HP, VT], BF16, tag="v",
                                    name=f"v{img}")

            def emit_unit(img, h, r0, rpt, pool_unit, dma_eng, ots,
                          phase_order=(0, 1, 2, 3)):
                """One work unit: rows [r0, r0+rpt) of C_out half h."""
                vt_ = vts[img]
                tf = rpt * VT
                ms = [None] * NPH
                for p in phase_order:
                    mp = psum.tile([128, 512], F32, tag=f"M{p}")
                    for kh in range(KH):
                        nc.tensor.matmul(
                            mp[:, :tf],
                            lhsT=w_sb[:, h, p * KH + kh],
                            rhs=vt_[:, p, r0 + kh:r0 + kh + rpt],
                            start=(kh == 0),
                            stop=(kh == KH - 1),
                        )
                    ms[p] = mp
                # even = M0+(M1+b)+M2, odd = (M1+b)-M2-M3. gpsimd has
                # no PSUM port, so "pool units" get all four phases
                # evacuated by ACT (e1 = M1+b via Identity bias) and
                # combine SBUF-side; the rest combine on DVE reading
                # PSUM directly. 3:5 split keeps every engine < ~80%.
                nm = f"{img}_{r0}_{h}"
                e1 = evac.tile([128, TF], F32, tag="e1", name=f"e1_{nm}")
                nc.scalar.activation(out=e1[:, :tf], in_=ms[1][:, :tf],
                                     func=IDENT, bias=b_sb[:, h:h + 1])
                ye = ots[h][:, r0:r0 + rpt, :, 0]
                yo = ots[h][:, r0:r0 + rpt, :, 1]
                if pool_unit:
                    e0 = evac.tile([128, TF], F32, tag="e0", name=f"e0_{nm}")
                    nc.scalar.activation(out=e0[:, :tf], in_=ms[0][:, :tf],
                                         func=COPY)
                    e2 = evac.tile([128, TF], F32, tag="e2", name=f"e2_{nm}")
                    nc.scalar.activation(out=e2[:, :tf], in_=ms[2][:, :tf],
                                         func=COPY)
                    e3 = evac.tile([128, TF], F32, tag="e3", name=f"e3_{nm}")
                    nc.scalar.activation(out=e3[:, :tf], in_=ms[3][:, :tf],
                                         func=COPY)
                    tmp1 = tmpp.tile([128, TF], F32, tag="tmp1",
                                     name=f"t1_{nm}")
                    nc.gpsimd.tensor_tensor(
                        out=tmp1[:, :tf], in0=e0[:, :tf], in1=e1[:, :tf],
                        op=ADD)
                    nc.gpsimd.tensor_tensor(
                        out=ye, in0=tmp1[:, :tf], in1=e2[:, :tf], op=ADD)
                    tmp2 = tmpp.tile([128, TF], F32, tag="tmp2",
                                     name=f"t2_{nm}")
                    nc.gpsimd.tensor_tensor(
                        out=tmp2[:, :tf], in0=e1[:, :tf], in1=e2[:, :tf],
                        op=SUB)
                    nc.gpsimd.tensor_tensor(
                        out=yo, in0=tmp2[:, :tf], in1=e3[:, :tf], op=SUB)
                else:
                    tmp1 = tmpp.tile([128, TF], F32, tag="tmp1",
                                     name=f"t1_{nm}")
                    nc.vector.tensor_tensor(
                        out=tmp1[:, :tf], in0=ms[0][:, :tf], in1=e1[:, :tf],
                        op=ADD)
                    nc.vector.tensor_tensor(
                        out=ye, in0=tmp1[:, :tf], in1=ms[2][:, :tf], op=ADD)
                    tmp2 = tmpp.tile([128, TF], F32, tag="tmp2",
                                     name=f"t2_{nm}")
                    nc.vector.tensor_tensor(
                        out=tmp2[:, :tf], in0=e1[:, :tf], in1=ms[2][:, :tf],
                        op=SUB)
                    nc.vector.tensor_tensor(
                        out=yo, in0=tmp2[:, :tf], in1=ms[3][:, :tf], op=SUB)
                dma_eng.dma_start(
                    out=out_v[img, h * 128:(h + 1) * 128,
                              r0 * W:(r0 + rpt) * W],
                    in_=ots[h][:, r0:r0 + rpt])

            for img in range(IMGS):
                ots = [outp.tile([128, H, VT, 2], F32, tag=f"ot{h}",
                                 name=f"ot{img}_{h}")
                       for h in range(NH)]
                if img == 0:
                    order = [(t, h) for h in range(NH) for t in range(NT)]
                else:
                    order = [(t, h) for t in range(NT) for h in range(NH)]
                for u, (t, h) in enumerate(order):
                    last_img = img == IMGS - 1
                    pool_unit = (u in POOL_UNITS if not last_img
                                 else u in (1, 3, 6))
                    eng = nc.scalar if (last_img and u == 7) else nc.sync
                    # last unit: M1 first so e1 (which gates every DVE
                    # combine) is ready early; only yo trails the last MM
                    porder = ((1, 0, 2, 3) if (last_img and u == 7)
                              else (0, 1, 2, 3))
                    emit_unit(img, h, RPT * t, RPT, pool_unit, eng, ots,
                              phase_order=porder)
                    if img == 0 and u in (0, 2, 4):
                        nxt = u // 2 + 1
                        nc.sync.dma_start(out=vts[nxt][:], in_=v[nxt])
    nc.compile()
    return nc


def get_program():
    if "nc" not in _CACHE:
        _CACHE["nc"] = _build_program()
    return _CACHE["nc"]


def make_in_maps(x, weight, bias):
    import ml_dtypes

    BF = ml_dtypes.bfloat16
    x = np.asarray(x, dtype=np.float32)
    weight = np.asarray(weight, dtype=np.float32)
    bias = np.asarray(bias, dtype=np.float32)

    xpad = np.zeros((N, C_IN, HP, WP), dtype=np.float32)
    xpad[:, :, 1:1 + H, 1:1 + W] = x
    xb = xpad.astype(BF).astype(np.float32)
    d0 = xb[:, :, :, 0:2 * VT:2]
    d1 = xb[:, :, :, 1:2 * VT + 1:2]
    d2 = xb[:, :, :, 2:2 * VT + 2:2]
    d3 = xb[:, :, :, 3:2 * VT + 2:2]
    V = np.stack([d0 - d2, d1 + d2, d2 - d1, d1 - d3], axis=2).astype(BF)
    V = np.ascontiguousarray(V)  # [N, C, 4, 58, 28]

    # U weights: [NH, C, p*3+kh, 128]
    U = np.stack([weight[:, :, :, 0],
                  (weight[:, :, :, 0] + weight[:, :, :, 1]
                   + weight[:, :, :, 2]) * 0.5,
                  (weight[:, :, :, 0] - weight[:, :, :, 1]
                   + weight[:, :, :, 2]) * 0.5,
                  weight[:, :, :, 2]], axis=2)  # [CO, C, 4, 3kh]
    U = U.transpose(1, 2, 3, 0).reshape(C_IN, NPH * KH, NH, 128)
    U = np.ascontiguousarray(U.transpose(2, 0, 1, 3).astype(BF))
    b2 = np.ascontiguousarray(bias.reshape(NH, 128).T)

    return [
        {
            "v": np.ascontiguousarray(V[i * IMGS:(i + 1) * IMGS]),
            "w": U,
            "b": b2,
        }
        for i in range(N_CORES)
    ]


def kernel(x, weight, bias):
    from concourse.bass_utils import run_bass_kernel_spmd

    nc = get_program()
    in_maps = make_in_maps(x, weight, bias)
    res = run_bass_kernel_spmd(nc, in_maps, core_ids=list(range(N_CORES)))
    return np.concatenate([res.results[i]["out"] for i in range(N_CORES)],
                          axis=0)
